# revision 45
# baseline (speedup 1.0000x reference)
"""DDiT block kernel for 8 Trainium2 NeuronCores.

Strategy: sequence parallel. Packed ragged segments (runs of equal seq_idx)
are assigned whole to cores (attention is block-diagonal, so no cross-core
communication is needed). Each core runs the full transformer block on its
padded local token slab with replicated, adaLN-modulation-folded bf16
weights. The block-diagonal mask is realized by appending one-hot segment-id
columns to q/k so masked scores come out 1024 below unmasked ones; exp with
bias -1024 then zeroes them exactly.

Host-side prep (cheap, O(D^2) on one row): ada = c @ w_ada + b_ada, folding
of LN scales / shifts / gates into the weight matrices, shard planning,
padding, one-hot build, RoPE table tiling.
"""

import math
from contextlib import ExitStack

import numpy as np
import ml_dtypes

import concourse.bass as bass
import concourse.tile as tile
from concourse import bacc, mybir
from concourse import bass_utils

BF16 = ml_dtypes.bfloat16

P = 128
DIM = 1024
HEADS = 16
HD = 64            # head dim
HHD = 32           # half head dim (rotary)
QKV = 3 * DIM
MLP = 4 * DIM
N_CORES = 8
OH = 16            # one-hot slots (8 segment ids + 1 padding id, padded to 16)
PAD_ID = 8
BIG = 1024.0
EPS = 1e-5

f32 = mybir.dt.float32
bf16 = mybir.dt.bfloat16
AF = mybir.ActivationFunctionType
ALU = mybir.AluOpType


def _splits(total, cap=512):
    """[(off, size)] chunks of at most cap."""
    out = []
    off = 0
    while off < total:
        out.append((off, min(cap, total - off)))
        off += cap
    return out


# --------------------------------------------------------------------------
# device kernel builder
# --------------------------------------------------------------------------

DEBUG_DUMPS = False


def _build(TLOC):
    assert TLOC % P == 0
    NT = TLOC // P

    nc = bacc.Bacc("TRN2", target_bir_lowering=False, debug=False,
                   num_devices=N_CORES)

    x_d = nc.dram_tensor("x", [TLOC, DIM], f32, kind="ExternalInput").ap()
    cq_d = nc.dram_tensor("cq", [TLOC, HHD * HEADS], bf16, kind="ExternalInput").ap()
    sq_d = nc.dram_tensor("sq", [TLOC, HHD * HEADS], bf16, kind="ExternalInput").ap()
    ck_d = nc.dram_tensor("ck", [TLOC, HHD * HEADS], bf16, kind="ExternalInput").ap()
    sk_d = nc.dram_tensor("sk", [TLOC, HHD * HEADS], bf16, kind="ExternalInput").ap()
    qex_d = nc.dram_tensor("qex", [TLOC, OH * HEADS], bf16, kind="ExternalInput").ap()
    kex_d = nc.dram_tensor("kex", [TLOC, OH * HEADS], bf16, kind="ExternalInput").ap()
    wqkv_d = nc.dram_tensor("wqkv", [DIM, QKV], bf16, kind="ExternalInput").ap()
    bqkv_d = nc.dram_tensor("bqkv", [QKV], f32, kind="ExternalInput").ap()
    wout_d = nc.dram_tensor("wout", [DIM, DIM], bf16, kind="ExternalInput").ap()
    wm1_d = nc.dram_tensor("wm1", [DIM, MLP], bf16, kind="ExternalInput").ap()
    # bm1 passed pre-transposed [128, MLP//128] so the load is contiguous
    bm1_d = nc.dram_tensor("bm1", [P, MLP // P], f32, kind="ExternalInput").ap()
    wm2_d = nc.dram_tensor("wm2", [MLP, DIM], bf16, kind="ExternalInput").ap()
    bm2_d = nc.dram_tensor("bm2", [DIM], bf16, kind="ExternalInput").ap()
    out_d = nc.dram_tensor("out", [TLOC, DIM], f32, kind="ExternalOutput").ap()
    dbg = {}
    if DEBUG_DUMPS:
        for name, shape in [("z_dbg", [TLOC, DIM]), ("qpad_dbg", [TLOC, HEADS * P]),
                            ("kpad_dbg", [TLOC, HEADS * P]), ("o_dbg", [TLOC, DIM]),
                            ("z2_dbg", [TLOC, DIM]), ("et_dbg", [TLOC, TLOC])]:
            dbg[name] = nc.dram_tensor(name, shape, bf16, kind="ExternalOutput").ap()
        dbg["po_dbg"] = nc.dram_tensor("po_dbg", [TLOC, HD + 1], f32,
                                       kind="ExternalOutput").ap()
        dbg["v_dbg"] = nc.dram_tensor("v_dbg", [TLOC, HEADS * (HD + 1)], bf16,
                                      kind="ExternalOutput").ap()

    NKD = DIM // P          # 8 contraction chunks over model dim
    NKM = MLP // P          # 32 contraction chunks over mlp dim
    NQN = QKV // 512        # 6 qkv output chunks of 512
    tq_splits = _splits(TLOC)    # moving-dim chunks of <=512

    with tile.TileContext(nc) as tc:
        with ExitStack() as ctx:
            dram = ctx.enter_context(tc.tile_pool(name="dram", bufs=1, space="DRAM"))
            z_dram = dram.tile([TLOC, DIM], bf16, name="z_dram")
            z2_dram = dram.tile([TLOC, DIM], bf16, name="z2_dram")
            qpad_dram = dram.tile([TLOC, HEADS * P], bf16, name="qpad_dram")
            kpad_dram = dram.tile([TLOC, HEADS * P], bf16, name="kpad_dram")
            o_dram = dram.tile([TLOC, DIM], bf16, name="o_dram")

            consts = ctx.enter_context(tc.tile_pool(name="consts", bufs=1))
            spool = ctx.enter_context(tc.tile_pool(name="spool", bufs=8))

            bias_bc = consts.tile([P, QKV], f32, name="bias_bc")
            nc.gpsimd.dma_start(
                out=bias_bc[:],
                in_=bqkv_d.rearrange("(o n) -> o n", o=1).to_broadcast([P, QKV]))
            bm1_sb = consts.tile([P, NKM], f32, name="bm1_sb")
            nc.gpsimd.dma_start(out=bm1_sb[:], in_=bm1_d[:, :])
            bm2_sb = consts.tile([1, DIM], bf16, name="bm2_sb")
            nc.gpsimd.dma_start(out=bm2_sb[:],
                                in_=bm2_d.rearrange("(o d) -> o d", o=1))
            ones_row = consts.tile([1, P], bf16, name="ones_row")
            nc.vector.memset(ones_row[:], 1.0)
            ones_col = consts.tile([P, 1], bf16, name="ones_col")
            nc.vector.memset(ones_col[:], 1.0)
            eps_t = consts.tile([P, 1], f32, name="eps_t")
            nc.vector.memset(eps_t[:], EPS)
            negbig_t = consts.tile([P, 1], f32, name="negbig_t")
            nc.vector.memset(negbig_t[:], -BIG)

            def ln_normalize(xt, z_out):
                """z_out = (xt - mean) * rsqrt(var + eps), row-wise over DIM."""
                xg = xt[:].rearrange("p (g d) -> p g d", d=512)
                stats = spool.tile([P, 2, 6], f32, tag="bnstats", name="bnstats")
                for g in range(2):
                    nc.vector.bn_stats(out=stats[:, g, :], in_=xg[:, g, :])
                mv = spool.tile([P, 2], f32, tag="bnmv", name="bnmv")
                nc.vector.bn_aggr(out=mv[:], in_=stats[:])
                rstd = spool.tile([P, 1], f32, tag="rstd", name="rstd")
                nc.scalar.activation(out=rstd[:], in_=mv[:, 1:2], func=AF.Sqrt,
                                     bias=eps_t[:], scale=1.0)
                nc.vector.reciprocal(out=rstd[:], in_=rstd[:])
                nc.vector.tensor_scalar(out=z_out[:], in0=xt[:],
                                        scalar1=mv[:, 0:1], scalar2=rstd[:],
                                        op0=ALU.subtract, op1=ALU.mult)

            # ---------------- phase 1: LN1 -> z -> z_dram -----------------
            xpool = ctx.enter_context(tc.tile_pool(name="xpool", bufs=NT + 2))
            x_sb = []
            with tc.tile_pool(name="zpool", bufs=3) as zpool:
                for c in range(NT):
                    xt = xpool.tile([P, DIM], f32, tag="x", name=f"x{c}")
                    nc.gpsimd.dma_start(out=xt[:], in_=x_d[c * P:(c + 1) * P, :])
                    x_sb.append(xt)
                    z = zpool.tile([P, DIM], bf16, tag="z", name=f"z{c}")
                    ln_normalize(xt, z)
                    nc.gpsimd.dma_start(out=z_dram[c * P:(c + 1) * P, :], in_=z[:])

            # ---------------- phase 2: qkv matmul + rope ------------------
            p23 = ctx.enter_context(ExitStack())
            vpool = p23.enter_context(tc.tile_pool(name="vpool", bufs=NT))
            v_sb = []
            with ExitStack() as qctx:
                ztpool = qctx.enter_context(tc.tile_pool(name="ztpool", bufs=NKD))
                zT = []
                for f in range(NKD):
                    t = ztpool.tile([P, TLOC], bf16, tag="zT", name=f"zT{f}")
                    nc.sync.dma_start(out=t[:], in_=z_dram[:, f * P:(f + 1) * P],
                                      transpose=True)
                    zT.append(t)

                wq_pool = qctx.enter_context(tc.tile_pool(name="wqkv", bufs=NKD * 3))
                wq = {}
                for k in range(NKD):
                    for n in range(3):
                        wt = wq_pool.tile([P, DIM], bf16, tag="wqkv",
                                          name=f"wqkv{k}_{n}")
                        nc.gpsimd.dma_start(
                            out=wt[:],
                            in_=wqkv_d[k * P:(k + 1) * P, n * DIM:(n + 1) * DIM])
                        wq[(k, n)] = wt

                rope_pool = qctx.enter_context(tc.tile_pool(name="rope", bufs=6))
                tpool = qctx.enter_context(tc.tile_pool(name="ropetmp", bufs=10))
                qkpre = qctx.enter_context(tc.tile_pool(name="qkpre", bufs=3))
                padpool = qctx.enter_context(tc.tile_pool(name="padpool", bufs=4))
                expool = qctx.enter_context(tc.tile_pool(name="expool", bufs=3))
                qkv_ps = qctx.enter_context(
                    tc.tile_pool(name="qkvps", bufs=6, space="PSUM"))

                for c in range(NT):
                    qpre = qkpre.tile([P, DIM], bf16, tag="qpre", name=f"qpre{c}")
                    kpre = qkpre.tile([P, DIM], bf16, tag="kpre", name=f"kpre{c}")
                    # v laid out [heads, 65] per token: col 64 of each head
                    # block is the ones column so a single matmul per chunk
                    # produces both o and the softmax denominator (two
                    # accumulation groups must not share a PSUM bank).
                    vt = vpool.tile([P, HEADS * (HD + 1)], bf16, tag="v",
                                    name=f"v{c}")
                    v_sb.append(vt)
                    vview = vt[:].rearrange("p (h e) -> p h e", e=HD + 1)
                    for hh in range(HEADS):
                        nc.vector.memset(vview[:, hh, HD:HD + 1], 1.0)
                    dsts = [(qpre, 0, 0), (qpre, 512, 0), (kpre, 0, 0),
                            (kpre, 512, 0), (vt, 0, 1), (vt, 0, 2)]
                    for nh in range(2):
                        ps = [qkv_ps.tile([P, 512], f32, tag="qkvps",
                                          name=f"qkvps{c}_{nh}_{i}")
                              for i in range(3)]
                        for k in range(NKD):
                            for i in range(3):
                                n = nh * 3 + i
                                nc.tensor.matmul(
                                    ps[i][:],
                                    lhsT=zT[k][:, c * P:(c + 1) * P],
                                    rhs=wq[(k, n // 2)][:, (n % 2) * 512:
                                                        (n % 2 + 1) * 512],
                                    start=(k == 0), stop=(k == NKD - 1))
                        for i in range(3):
                            n = nh * 3 + i
                            dst, off, vh = dsts[n]
                            if vh == 0:
                                out_ap = dst[:, off:off + 512]
                            else:
                                out_ap = vview[:, (vh - 1) * 8:vh * 8, 0:HD]
                            nc.vector.tensor_add(
                                out=out_ap, in0=ps[i][:],
                                in1=bias_bc[:, n * 512:(n + 1) * 512])

                    # rope for q and k of this token chunk, staged through a
                    # padded SBUF tile then one contiguous DMA to DRAM
                    for (pre, cos_d, sin_d, ex_d, pad_dst) in (
                            (qpre, cq_d, sq_d, qex_d, qpad_dram),
                            (kpre, ck_d, sk_d, kex_d, kpad_dram)):
                        cos_t = rope_pool.tile([P, HHD * HEADS], bf16,
                                               tag="cos", name=f"cos{c}")
                        sin_t = rope_pool.tile([P, HHD * HEADS], bf16,
                                               tag="sin", name=f"sin{c}")
                        nc.gpsimd.dma_start(out=cos_t[:],
                                            in_=cos_d[c * P:(c + 1) * P, :])
                        nc.gpsimd.dma_start(out=sin_t[:],
                                            in_=sin_d[c * P:(c + 1) * P, :])
                        ex_t = expool.tile([P, OH * HEADS], bf16, tag="ex",
                                           name=f"ex{c}")
                        nc.gpsimd.dma_start(out=ex_t[:],
                                            in_=ex_d[c * P:(c + 1) * P, :])
                        pad_t = padpool.tile([P, HEADS * P], bf16, tag="pad",
                                             name=f"pad{c}")
                        pv = pad_t[:].rearrange("p (h d) -> p h d", d=P)
                        hv = pre[:].rearrange("p (h d) -> p h d", d=HD)
                        h1 = hv[:, :, 0:HHD]
                        h2 = hv[:, :, HHD:HD]
                        cv = cos_t[:].rearrange("p (h d) -> p h d", d=HHD)
                        sv = sin_t[:].rearrange("p (h d) -> p h d", d=HHD)
                        ta = tpool.tile([P, HHD * HEADS], bf16, tag="ta", name="ta")
                        tb = tpool.tile([P, HHD * HEADS], bf16, tag="tb", name="tb")
                        tcx = tpool.tile([P, HHD * HEADS], bf16, tag="tc", name="tc")
                        tav = ta[:].rearrange("p (h d) -> p h d", d=HHD)
                        tbv = tb[:].rearrange("p (h d) -> p h d", d=HHD)
                        tcv = tcx[:].rearrange("p (h d) -> p h d", d=HHD)
                        nc.vector.tensor_mul(tav, h1, cv)
                        nc.vector.tensor_mul(tbv, h2, sv)
                        nc.vector.tensor_sub(pv[:, :, 0:HHD], tav, tbv)
                        nc.vector.tensor_mul(tcv, h1, sv)
                        nc.vector.tensor_mul(tav, h2, cv)
                        nc.vector.tensor_add(pv[:, :, HHD:HD], tcv, tav)
                        nc.vector.tensor_copy(
                            out=pv[:, :, HD:HD + OH],
                            in_=ex_t[:].rearrange("p (h e) -> p h e", e=OH))
                        nc.gpsimd.dma_start(
                            out=pad_dst[c * P:(c + 1) * P, :], in_=pad_t[:])

            # ---------------- phase 3: attention --------------------------
            opool = p23.enter_context(tc.tile_pool(name="opool", bufs=NT))
            o_sb = [opool.tile([P, DIM], bf16, tag="o", name=f"o{m}")
                    for m in range(NT)]
            with ExitStack() as actx:
                qt_pool = actx.enter_context(tc.tile_pool(name="qtp", bufs=8))
                et_pool = actx.enter_context(tc.tile_pool(name="etp", bufs=3 * NT))
                sc_ps = actx.enter_context(
                    tc.tile_pool(name="scps", bufs=2, space="PSUM"))
                av_ps = actx.enter_context(
                    tc.tile_pool(name="avps", bufs=4, space="PSUM"))
                for h in range(HEADS):
                    qT = qt_pool.tile([P, TLOC], bf16, tag="qT", name=f"qT{h}")
                    kT = qt_pool.tile([P, TLOC], bf16, tag="kT", name=f"kT{h}")
                    nc.sync.dma_start(out=qT[:], in_=qpad_dram[:, h * P:(h + 1) * P],
                                      transpose=True)
                    nc.sync.dma_start(out=kT[:], in_=kpad_dram[:, h * P:(h + 1) * P],
                                      transpose=True)
                    eT = []
                    for cts in range(NT):
                        ps = sc_ps.tile([P, TLOC], f32, tag="scps",
                                        name=f"scps{h}_{cts}")
                        for (off, size) in tq_splits:
                            nc.tensor.matmul(
                                ps[:, off:off + size],
                                lhsT=kT[0:HD + OH, cts * P:(cts + 1) * P],
                                rhs=qT[0:HD + OH, off:off + size],
                                start=True, stop=True)
                        et = et_pool.tile([P, TLOC], bf16, tag="eT",
                                          name=f"eT{h}_{cts}")
                        nc.scalar.activation(out=et[:], in_=ps[:], func=AF.Exp,
                                             bias=negbig_t[:], scale=1.0)
                        if DEBUG_DUMPS and h == 0:
                            nc.sync.dma_start(
                                out=dbg["et_dbg"][cts * P:(cts + 1) * P, :],
                                in_=et[:])
                        eT.append(et)
                    for m in range(NT):
                        po = av_ps.tile([P, HD + 1], f32, tag="avps",
                                        name=f"avps{h}_{m}")
                        for cts in range(NT):
                            nc.tensor.matmul(
                                po[:],
                                lhsT=eT[cts][:, m * P:(m + 1) * P],
                                rhs=v_sb[cts][:, h * (HD + 1):(h + 1) * (HD + 1)],
                                start=(cts == 0), stop=(cts == NT - 1))
                        rd = spool.tile([P, 1], f32, tag="rd", name=f"rd{h}_{m}")
                        if DEBUG_DUMPS and h == 0:
                            pocp = spool.tile([P, HD + 1], f32, tag="pocp",
                                              name=f"pocp{m}")
                            nc.vector.tensor_copy(out=pocp[:], in_=po[:])
                            nc.sync.dma_start(
                                out=dbg["po_dbg"][m * P:(m + 1) * P, :],
                                in_=pocp[:])
                        nc.vector.reciprocal(out=rd[:], in_=po[:, HD:HD + 1])
                        nc.vector.tensor_scalar_mul(
                            out=o_sb[m][:, h * HD:(h + 1) * HD],
                            in0=po[:, 0:HD], scalar1=rd[:])
            for m in range(NT):
                nc.gpsimd.dma_start(out=o_dram[m * P:(m + 1) * P, :], in_=o_sb[m][:])
            if DEBUG_DUMPS:
                for c in range(NT):
                    nc.sync.dma_start(out=dbg["v_dbg"][c * P:(c + 1) * P, :],
                                      in_=v_sb[c][:])
            p23.close()

            # ------------- phase 4: out proj + residual + LN2 -------------
            x2_sb = []
            with ExitStack() as octx:
                ot_pool = octx.enter_context(tc.tile_pool(name="otp", bufs=NKD))
                oT = []
                for f in range(NKD):
                    t = ot_pool.tile([P, TLOC], bf16, tag="oT", name=f"oT{f}")
                    nc.sync.dma_start(out=t[:], in_=o_dram[:, f * P:(f + 1) * P],
                                      transpose=True)
                    oT.append(t)
                wo_pool = octx.enter_context(tc.tile_pool(name="wout", bufs=NKD))
                wo = {}
                for k in range(NKD):
                    wt = wo_pool.tile([P, DIM], bf16, tag="wout",
                                      name=f"wout{k}")
                    nc.gpsimd.dma_start(out=wt[:], in_=wout_d[k * P:(k + 1) * P, :])
                    wo[k] = wt
                op_ps = octx.enter_context(
                    tc.tile_pool(name="opps", bufs=4, space="PSUM"))
                with tc.tile_pool(name="z2pool", bufs=3) as z2pool:
                    for m in range(NT):
                        ps = [op_ps.tile([P, 512], f32, tag="opps",
                                         name=f"opps{m}_{n}") for n in range(2)]
                        for k in range(NKD):
                            for n in range(2):
                                nc.tensor.matmul(
                                    ps[n][:],
                                    lhsT=oT[k][:, m * P:(m + 1) * P],
                                    rhs=wo[k][:, n * 512:(n + 1) * 512],
                                    start=(k == 0), stop=(k == NKD - 1))
                        x2 = xpool.tile([P, DIM], f32, tag="x", name=f"x2_{m}")
                        for n in range(2):
                            nc.vector.tensor_add(
                                out=x2[:, n * 512:(n + 1) * 512], in0=ps[n][:],
                                in1=x_sb[m][:, n * 512:(n + 1) * 512])
                        x2_sb.append(x2)
                        z2 = z2pool.tile([P, DIM], bf16, tag="z2", name=f"z2_{m}")
                        ln_normalize(x2, z2)
                        nc.gpsimd.dma_start(out=z2_dram[m * P:(m + 1) * P, :],
                                            in_=z2[:])

            # ---------------- phase 5: MLP --------------------------------
            with ExitStack() as mctx:
                z2t_pool = mctx.enter_context(tc.tile_pool(name="z2tp", bufs=NKD))
                z2T = []
                for f in range(NKD):
                    t = z2t_pool.tile([P, TLOC], bf16, tag="z2T", name=f"z2T{f}")
                    nc.sync.dma_start(out=t[:], in_=z2_dram[:, f * P:(f + 1) * P],
                                      transpose=True)
                    z2T.append(t)
                gh_pool = mctx.enter_context(tc.tile_pool(name="ghp", bufs=NKM))
                wm1_pool = mctx.enter_context(tc.tile_pool(name="wm1p", bufs=10))
                m1_ps = mctx.enter_context(
                    tc.tile_pool(name="m1ps", bufs=2, space="PSUM"))
                ghT = []
                for mw in range(NKM // 8):
                    wts = {}
                    for k in range(NKD):
                        wt = wm1_pool.tile([P, DIM], bf16, tag="wm1",
                                           name=f"wm1_{mw}_{k}")
                        nc.gpsimd.dma_start(
                            out=wt[:],
                            in_=wm1_d[k * P:(k + 1) * P, mw * DIM:(mw + 1) * DIM])
                        wts[k] = wt
                    for mi in range(8):
                        m = mw * 8 + mi
                        pm = m1_ps.tile([P, TLOC], f32, tag="m1ps", name=f"m1ps{m}")
                        for k in range(NKD):
                            for (off, size) in tq_splits:
                                nc.tensor.matmul(
                                    pm[:, off:off + size],
                                    lhsT=wts[k][:, mi * P:(mi + 1) * P],
                                    rhs=z2T[k][:, off:off + size],
                                    start=(k == 0), stop=(k == NKD - 1))
                        gh = gh_pool.tile([P, TLOC], bf16, tag="ghT", name=f"ghT{m}")
                        nc.scalar.activation(out=gh[:], in_=pm[:],
                                             func=AF.Gelu_apprx_tanh,
                                             bias=bm1_sb[:, m:m + 1], scale=1.0)
                        ghT.append(gh)

                wm2_pool = mctx.enter_context(tc.tile_pool(name="wm2p", bufs=NKM))
                wm2 = {}
                for k in range(NKM):
                    wt = wm2_pool.tile([P, DIM], bf16, tag="wm2", name=f"wm2_{k}")
                    nc.gpsimd.dma_start(out=wt[:], in_=wm2_d[k * P:(k + 1) * P, :])
                    wm2[k] = wt
                m2_ps = mctx.enter_context(
                    tc.tile_pool(name="m2ps", bufs=4, space="PSUM"))
                with tc.tile_pool(name="ostage", bufs=3) as ostage:
                    for m in range(NT):
                        ps = [m2_ps.tile([P, 512], f32, tag="m2ps",
                                         name=f"m2ps{m}_{n}") for n in range(2)]
                        for n in range(2):
                            nc.tensor.matmul(
                                ps[n][:], lhsT=ones_row[:],
                                rhs=bm2_sb[0:1, n * 512:(n + 1) * 512],
                                start=True, stop=False)
                        for k in range(NKM):
                            for n in range(2):
                                nc.tensor.matmul(
                                    ps[n][:],
                                    lhsT=ghT[k][:, m * P:(m + 1) * P],
                                    rhs=wm2[k][:, n * 512:(n + 1) * 512],
                                    start=False, stop=(k == NKM - 1))
                        ot = ostage.tile([P, DIM], f32, tag="ost", name=f"ost{m}")
                        for n in range(2):
                            nc.vector.tensor_add(
                                out=ot[:, n * 512:(n + 1) * 512], in0=ps[n][:],
                                in1=x2_sb[m][:, n * 512:(n + 1) * 512])
                        nc.gpsimd.dma_start(out=out_d[m * P:(m + 1) * P, :],
                                            in_=ot[:])

            if DEBUG_DUMPS:
                for name, src in [("z_dbg", z_dram), ("qpad_dbg", qpad_dram),
                                  ("kpad_dbg", kpad_dram), ("o_dbg", o_dram),
                                  ("z2_dbg", z2_dram)]:
                    nc.sync.dma_start(out=dbg[name][:, :], in_=src[:, :])

    nc.compile()
    return nc


_NC_CACHE = {}


def _get_nc(TLOC):
    if TLOC not in _NC_CACHE:
        _NC_CACHE[TLOC] = _build(TLOC)
    return _NC_CACHE[TLOC]


# --------------------------------------------------------------------------
# host-side prep
# --------------------------------------------------------------------------

def _partition_contiguous(sizes, k):
    """Split list of segment sizes into <=k contiguous groups minimizing the
    max group total. Returns list of (start_seg, end_seg) pairs."""
    n = len(sizes)
    prefix = np.concatenate([[0], np.cumsum(sizes)])

    def feasible(cap):
        groups = 0
        i = 0
        while i < n:
            if sizes[i] > cap:
                return None
            j = i
            while j < n and prefix[j + 1] - prefix[i] <= cap:
                j += 1
            groups += 1
            i = j
        return groups

    lo, hi = int(max(sizes)), int(prefix[-1])
    while lo < hi:
        mid = (lo + hi) // 2
        g = feasible(mid)
        if g is not None and g <= k:
            hi = mid
        else:
            lo = mid + 1
    cap = lo
    out = []
    i = 0
    while i < n:
        j = i
        while j < n and prefix[j + 1] - prefix[i] <= cap:
            j += 1
        out.append((i, j))
        i = j
    return out


def _prep(inputs):
    x = np.asarray(inputs["x"], np.float32)[0]          # [T, D]
    cvec = np.asarray(inputs["c"], np.float32)          # [1, COND]
    cos = np.asarray(inputs["cos"], np.float32)         # [T, 32]
    sin = np.asarray(inputs["sin"], np.float32)
    seq = np.asarray(inputs["seq_idx"]).astype(np.int64)
    ln1_w = np.asarray(inputs["ln1_w"], np.float32)
    ln1_b = np.asarray(inputs["ln1_b"], np.float32)
    w_qkv = np.asarray(inputs["w_qkv"], np.float32)
    w_out = np.asarray(inputs["w_out"], np.float32)
    ln2_w = np.asarray(inputs["ln2_w"], np.float32)
    ln2_b = np.asarray(inputs["ln2_b"], np.float32)
    w_mlp1 = np.asarray(inputs["w_mlp1"], np.float32)
    b_mlp1 = np.asarray(inputs["b_mlp1"], np.float32)
    w_mlp2 = np.asarray(inputs["w_mlp2"], np.float32)
    b_mlp2 = np.asarray(inputs["b_mlp2"], np.float32)
    w_ada = np.asarray(inputs["w_ada"], np.float32)
    b_ada = np.asarray(inputs["b_ada"], np.float32)

    T = x.shape[0]

    ada = (cvec @ w_ada + b_ada)[0]                     # [6*DIM]
    sh_msa, sc_msa, g_msa, sh_mlp, sc_mlp, g_mlp = np.split(ada, 6)

    W1 = ln1_w * (1.0 + sc_msa)
    B1 = ln1_b * (1.0 + sc_msa) + sh_msa
    wqkv_f = (W1[:, None] * w_qkv).astype(BF16)
    bqkv_f = (B1 @ w_qkv).astype(np.float32)
    wout_f = (w_out * g_msa[None, :]).astype(BF16)
    W2 = ln2_w * (1.0 + sc_mlp)
    B2 = ln2_b * (1.0 + sc_mlp) + sh_mlp
    wm1_f = (W2[:, None] * w_mlp1).astype(BF16)
    bm1_f = (b_mlp1 + B2 @ w_mlp1).astype(np.float32).reshape(MLP // P, P).T.copy()
    wm2_f = (w_mlp2 * g_mlp[None, :]).astype(BF16)
    bm2_f = (b_mlp2 * g_mlp).astype(BF16)

    # segment runs
    bnd = np.flatnonzero(np.diff(seq)) + 1
    seg_starts = np.concatenate([[0], bnd]).astype(int)
    seg_ends = np.concatenate([bnd, [T]]).astype(int)
    sizes = (seg_ends - seg_starts).astype(int)
    groups = _partition_contiguous(sizes, N_CORES)
    tok_ranges = [(seg_starts[a], seg_ends[b - 1]) for (a, b) in groups]
    while len(tok_ranges) < N_CORES:
        tok_ranges.append((T, T))                       # empty shard
    max_n = max(e - s for (s, e) in tok_ranges)
    TLOC = max(P, P * math.ceil(max_n / P))

    in_maps = []
    for (s, e) in tok_ranges:
        n = e - s
        x_loc = np.zeros((TLOC, DIM), np.float32)
        x_loc[:n] = x[s:e]
        ids = np.full(TLOC, PAD_ID, np.int64)
        ids[:n] = seq[s:e]
        oh = np.zeros((TLOC, OH), np.float32)
        oh[np.arange(TLOC), ids] = 1.0
        cos_loc = np.zeros((TLOC, HHD), np.float32)
        sin_loc = np.zeros((TLOC, HHD), np.float32)
        cos_loc[:n] = cos[s:e]
        sin_loc[:n] = sin[s:e]
        scale = 1.0 / math.sqrt(HD)
        in_maps.append({
            "x": x_loc,
            "cq": np.tile(scale * cos_loc, (1, HEADS)).astype(BF16),
            "sq": np.tile(scale * sin_loc, (1, HEADS)).astype(BF16),
            "ck": np.tile(cos_loc, (1, HEADS)).astype(BF16),
            "sk": np.tile(sin_loc, (1, HEADS)).astype(BF16),
            "qex": np.tile(BIG * oh, (1, HEADS)).astype(BF16),
            "kex": np.tile(oh, (1, HEADS)).astype(BF16),
            "wqkv": wqkv_f, "bqkv": bqkv_f,
            "wout": wout_f,
            "wm1": wm1_f, "bm1": bm1_f,
            "wm2": wm2_f, "bm2": bm2_f,
        })
    return in_maps, tok_ranges, TLOC, T


def kernel(**inputs) -> np.ndarray:
    in_maps, tok_ranges, TLOC, T = _prep(inputs)
    nc = _get_nc(TLOC)
    res = bass_utils.run_bass_kernel_spmd(nc, in_maps,
                                          core_ids=list(range(N_CORES)))
    out = np.empty((T, DIM), np.float32)
    for core, (s, e) in enumerate(tok_ranges):
        if e > s:
            out[s:e] = res.results[core]["out"][:e - s]
    return out[None]


# revision 59
# speedup vs baseline: 1.3547x; 1.3547x over previous
"""DDiT block kernel for 8 Trainium2 NeuronCores.

Strategy: sequence parallel. Packed ragged segments (runs of equal seq_idx)
are assigned whole to cores (attention is block-diagonal, so no cross-core
communication is needed). Each core runs the full transformer block on its
padded local token slab with replicated, adaLN-modulation-folded bf16
weights. The block-diagonal mask is realized by appending one-hot segment-id
columns to q/k so masked scores come out 1024 below unmasked ones; exp with
bias -1024 then zeroes them exactly.

Host-side prep (cheap, O(D^2) on one row): ada = c @ w_ada + b_ada, folding
of LN scales / shifts / gates into the weight matrices, shard planning,
padding, one-hot build, RoPE table tiling.
"""

import math
from contextlib import ExitStack

import numpy as np
import ml_dtypes

import concourse.bass as bass
import concourse.tile as tile
from concourse import bacc, mybir
from concourse import bass_utils
from concourse import masks

BF16 = ml_dtypes.bfloat16

P = 128
DIM = 1024
HEADS = 16
HD = 64            # head dim
HHD = 32           # half head dim (rotary)
QKV = 3 * DIM
MLP = 4 * DIM
N_CORES = 8
OH = 16            # one-hot slots (8 segment ids + 1 padding id, padded to 16)
PAD_ID = 8
BIG = 1024.0
EPS = 1e-5

f32 = mybir.dt.float32
bf16 = mybir.dt.bfloat16
AF = mybir.ActivationFunctionType
ALU = mybir.AluOpType


def _splits(total, cap=512):
    """[(off, size)] chunks of at most cap."""
    out = []
    off = 0
    while off < total:
        out.append((off, min(cap, total - off)))
        off += cap
    return out


# --------------------------------------------------------------------------
# device kernel builder
# --------------------------------------------------------------------------

DEBUG_DUMPS = False


def _build(TLOC):
    assert TLOC % P == 0
    NT = TLOC // P

    nc = bacc.Bacc("TRN2", target_bir_lowering=False, debug=False,
                   num_devices=N_CORES)

    x_d = nc.dram_tensor("x", [TLOC, DIM], f32, kind="ExternalInput").ap()
    cq_d = nc.dram_tensor("cq", [TLOC, HHD * HEADS], bf16, kind="ExternalInput").ap()
    sq_d = nc.dram_tensor("sq", [TLOC, HHD * HEADS], bf16, kind="ExternalInput").ap()
    ck_d = nc.dram_tensor("ck", [TLOC, HHD * HEADS], bf16, kind="ExternalInput").ap()
    sk_d = nc.dram_tensor("sk", [TLOC, HHD * HEADS], bf16, kind="ExternalInput").ap()
    qex_d = nc.dram_tensor("qex", [TLOC, OH * HEADS], bf16, kind="ExternalInput").ap()
    kex_d = nc.dram_tensor("kex", [TLOC, OH * HEADS], bf16, kind="ExternalInput").ap()
    wqkv_d = nc.dram_tensor("wqkv", [DIM, QKV], bf16, kind="ExternalInput").ap()
    bqkv_d = nc.dram_tensor("bqkv", [QKV], bf16, kind="ExternalInput").ap()
    wout_d = nc.dram_tensor("wout", [DIM, DIM], bf16, kind="ExternalInput").ap()
    wm1_d = nc.dram_tensor("wm1", [DIM, MLP], bf16, kind="ExternalInput").ap()
    # bm1 passed pre-transposed [128, MLP//128] so the load is contiguous
    bm1_d = nc.dram_tensor("bm1", [P, MLP // P], f32, kind="ExternalInput").ap()
    wm2_d = nc.dram_tensor("wm2", [MLP, DIM], bf16, kind="ExternalInput").ap()
    bm2_d = nc.dram_tensor("bm2", [DIM], bf16, kind="ExternalInput").ap()
    out_d = nc.dram_tensor("out", [TLOC, DIM], f32, kind="ExternalOutput").ap()
    dbg = {}
    if DEBUG_DUMPS:
        for name, shape in [("z_dbg", [TLOC, DIM]), ("qpad_dbg", [TLOC, HEADS * P]),
                            ("kpad_dbg", [TLOC, HEADS * P]), ("o_dbg", [TLOC, DIM]),
                            ("z2_dbg", [TLOC, DIM]), ("et_dbg", [TLOC, TLOC]),
                            ("v_dbg", [TLOC, HEADS * (HD + 1)])]:
            dbg[name] = nc.dram_tensor(name, shape, bf16, kind="ExternalOutput").ap()
        dbg["po_dbg"] = nc.dram_tensor("po_dbg", [TLOC, HD + 1], f32,
                                       kind="ExternalOutput").ap()

    NKD = DIM // P          # 8 contraction chunks over model dim
    NKM = MLP // P          # 32 contraction chunks over mlp dim
    tq_splits = _splits(TLOC)    # moving-dim chunks of <=512

    with tile.TileContext(nc) as tc:
        with ExitStack() as ctx:
            consts = ctx.enter_context(tc.tile_pool(name="consts", bufs=1))
            spool = ctx.enter_context(tc.tile_pool(name="spool", bufs=8))
            x2pool = ctx.enter_context(tc.tile_pool(name="x2pool", bufs=NT))
            otzpool = ctx.enter_context(tc.tile_pool(name="otzpool", bufs=1))
            oT = otzpool.tile([P, NKD, TLOC], bf16, name="oT_all")
            z2T = otzpool.tile([P, NKD, TLOC], bf16, name="z2T_all")

            ident = consts.tile([P, P], bf16, name="ident")
            masks.make_identity(nc, ident[:])
            bm1_sb = consts.tile([P, NKM], f32, name="bm1_sb")
            nc.gpsimd.dma_start(out=bm1_sb[:], in_=bm1_d[:, :])
            ones_row = consts.tile([1, P], bf16, name="ones_row")
            nc.vector.memset(ones_row[:], 1.0)
            eps_t = consts.tile([P, 1], f32, name="eps_t")
            nc.vector.memset(eps_t[:], EPS)
            negbig_t = consts.tile([P, 1], f32, name="negbig_t")
            nc.vector.memset(negbig_t[:], -BIG)

            def ln_normalize(xt, z_out):
                """z_out = (xt - mean) * rsqrt(var + eps), row-wise over DIM."""
                xg = xt[:].rearrange("p (g d) -> p g d", d=512)
                stats = spool.tile([P, 2, 6], f32, tag="sc", name="bnstats")
                for g in range(2):
                    nc.vector.bn_stats(out=stats[:, g, :], in_=xg[:, g, :])
                mv = spool.tile([P, 2], f32, tag="sc", name="bnmv")
                nc.vector.bn_aggr(out=mv[:], in_=stats[:])
                rstd = spool.tile([P, 1], f32, tag="sc", name="rstd")
                nc.scalar.activation(out=rstd[:], in_=mv[:, 1:2], func=AF.Sqrt,
                                     bias=eps_t[:], scale=1.0)
                nc.vector.reciprocal(out=rstd[:], in_=rstd[:])
                nc.vector.tensor_scalar(out=z_out[:], in0=xt[:],
                                        scalar1=mv[:, 0:1], scalar2=rstd[:],
                                        op0=ALU.subtract, op1=ALU.mult)

            # pools that live from phase 1 through the out-projection
            p23 = ctx.enter_context(ExitStack())
            tps_pool = p23.enter_context(
                tc.tile_pool(name="tps", bufs=2, space="PSUM"))
            xpool = p23.enter_context(tc.tile_pool(name="xpool", bufs=NT))
            ztpool = p23.enter_context(tc.tile_pool(name="ztpool", bufs=1))
            zT = ztpool.tile([P, NKD, TLOC], bf16, name="zT_all")
            vpool = p23.enter_context(tc.tile_pool(name="vpool", bufs=NT))
            qktpool = p23.enter_context(tc.tile_pool(name="qktpool", bufs=1))
            qT = qktpool.tile([P, HEADS, TLOC], bf16, name="qT_all")
            kT = qktpool.tile([P, HEADS, TLOC], bf16, name="kT_all")
            opool = p23.enter_context(tc.tile_pool(name="opool", bufs=NT))

            def pe_transpose_into(dst_all, src_slices, cslice):
                """Transpose 8 [P, P] src slices into dst_all[:, f, cslice]
                via PE, packing 4 per PSUM bank before one strided evac."""
                for fg in range(2):
                    tps = tps_pool.tile([P, 4 * P], bf16, tag="tps", name="tps")
                    for fi in range(4):
                        nc.tensor.transpose(
                            tps[:, fi * P:(fi + 1) * P],
                            src_slices[fg * 4 + fi], ident[:])
                    nc.vector.tensor_copy(
                        out=dst_all[:, fg * 4:(fg + 1) * 4, cslice],
                        in_=tps[:].rearrange("p (g q) -> p g q", q=P))

            # ---------------- phase 1: LN1 -> z -> zT ---------------------
            x_sb = []
            with tc.tile_pool(name="zpool", bufs=3) as zpool:
                for c in range(NT):
                    xt = xpool.tile([P, DIM], f32, tag="x", name=f"x{c}")
                    nc.gpsimd.dma_start(out=xt[:], in_=x_d[c * P:(c + 1) * P, :])
                    x_sb.append(xt)
                    z = zpool.tile([P, DIM], bf16, tag="z", name=f"z{c}")
                    ln_normalize(xt, z)
                    if DEBUG_DUMPS:
                        nc.sync.dma_start(out=dbg["z_dbg"][c * P:(c + 1) * P, :],
                                          in_=z[:])
                    pe_transpose_into(
                        zT, [z[:, f * P:(f + 1) * P] for f in range(NKD)],
                        slice(c * P, (c + 1) * P))

            # ---------------- phase 2: qkv matmul + rope ------------------
            v_sb = []
            with ExitStack() as qctx:
                wq_pool = qctx.enter_context(tc.tile_pool(name="wqkv", bufs=12))
                bqpool = qctx.enter_context(tc.tile_pool(name="bqpool", bufs=2))
                rope_pool = qctx.enter_context(tc.tile_pool(name="rope", bufs=2))
                tpool = qctx.enter_context(tc.tile_pool(name="ropetmp", bufs=2))
                padpool = qctx.enter_context(tc.tile_pool(name="padpool",
                                                          bufs=NT))
                expool = qctx.enter_context(tc.tile_pool(name="expool", bufs=2))
                qkv_ps = qctx.enter_context(
                    tc.tile_pool(name="qkvps", bufs=4, space="PSUM"))

                qpad_sb, kpad_sb = [], []
                for c in range(NT):
                    # v laid out [heads, 65] per token: col 64 of each head
                    # block is a ones column so a single matmul per ts-chunk
                    # yields both o and the softmax denominator (two
                    # accumulation groups must not share a PSUM bank).
                    vt = vpool.tile([P, HEADS * (HD + 1)], bf16, tag="v",
                                    name=f"v{c}")
                    v_sb.append(vt)
                    vview = vt[:].rearrange("p (h e) -> p h e", e=HD + 1)
                    for hh in range(HEADS):
                        nc.vector.memset(vview[:, hh, HD:HD + 1], 1.0)

                def rope_chunk(c, pad_t, cos_d, sin_d, ex_d, dst_all, dbg_name):
                    """in-place rope + one-hot staging + PE transpose."""
                    cos_t = rope_pool.tile([P, HHD * HEADS], bf16,
                                           tag="cos", name=f"cos{c}")
                    sin_t = rope_pool.tile([P, HHD * HEADS], bf16,
                                           tag="sin", name=f"sin{c}")
                    nc.gpsimd.dma_start(out=cos_t[:],
                                        in_=cos_d[c * P:(c + 1) * P, :])
                    nc.gpsimd.dma_start(out=sin_t[:],
                                        in_=sin_d[c * P:(c + 1) * P, :])
                    ex_t = expool.tile([P, OH * HEADS], bf16, tag="ex",
                                       name=f"ex{c}")
                    nc.gpsimd.dma_start(out=ex_t[:],
                                        in_=ex_d[c * P:(c + 1) * P, :])
                    pv = pad_t[:].rearrange("p (h d) -> p h d", d=P)
                    h1 = pv[:, :, 0:HHD]
                    h2 = pv[:, :, HHD:HD]
                    cv = cos_t[:].rearrange("p (h d) -> p h d", d=HHD)
                    sv = sin_t[:].rearrange("p (h d) -> p h d", d=HHD)
                    ta = tpool.tile([P, HHD * HEADS], bf16, tag="ta", name="ta")
                    tb = tpool.tile([P, HHD * HEADS], bf16, tag="tb", name="tb")
                    tcx = tpool.tile([P, HHD * HEADS], bf16, tag="tc", name="tc")
                    td = tpool.tile([P, HHD * HEADS], bf16, tag="td", name="td")
                    tav = ta[:].rearrange("p (h d) -> p h d", d=HHD)
                    tbv = tb[:].rearrange("p (h d) -> p h d", d=HHD)
                    tcv = tcx[:].rearrange("p (h d) -> p h d", d=HHD)
                    tdv = td[:].rearrange("p (h d) -> p h d", d=HHD)
                    nc.vector.tensor_mul(tav, h1, cv)
                    nc.vector.tensor_mul(tbv, h2, sv)
                    nc.vector.tensor_mul(tcv, h1, sv)
                    nc.vector.tensor_mul(tdv, h2, cv)
                    nc.vector.tensor_sub(h1, tav, tbv)
                    nc.vector.tensor_add(h2, tcv, tdv)
                    nc.vector.tensor_copy(
                        out=pv[:, :, HD:HD + OH],
                        in_=ex_t[:].rearrange("p (h e) -> p h e", e=OH))
                    if DEBUG_DUMPS:
                        nc.sync.dma_start(
                            out=dbg[dbg_name][c * P:(c + 1) * P, :],
                            in_=pad_t[:])
                    for hg in range(4):
                        tps = tps_pool.tile([P, 4 * P], bf16, tag="tps",
                                            name="tps")
                        for hi in range(4):
                            h = hg * 4 + hi
                            nc.tensor.transpose(
                                tps[:, hi * P:(hi + 1) * P],
                                pad_t[:, h * P:(h + 1) * P], ident[:])
                        nc.vector.tensor_copy(
                            out=dst_all[:, hg * 4:(hg + 1) * 4,
                                        c * P:(c + 1) * P],
                            in_=tps[:].rearrange("p (g q) -> p g q", q=P))

                bqv = bqkv_d.rearrange("(o n) -> o n", o=1)
                for n in range(6):
                    bq_t = bqpool.tile([1, 512], bf16, tag="bq", name=f"bq{n}")
                    nc.gpsimd.dma_start(out=bq_t[:],
                                        in_=bqv[0:1, n * 512:(n + 1) * 512])
                    wts = {}
                    for k in range(NKD):
                        wt = wq_pool.tile([P, 512], bf16, tag="wqkv",
                                          name=f"wqkv{n}_{k}")
                        nc.gpsimd.dma_start(
                            out=wt[:],
                            in_=wqkv_d[k * P:(k + 1) * P, n * 512:(n + 1) * 512])
                        wts[k] = wt
                    for c in range(NT):
                        if n == 0:
                            qpad_sb.append(padpool.tile([P, HEADS * P], bf16,
                                                        tag="pad", name=f"qpad{c}"))
                        if n == 2:
                            kpad_sb.append(padpool.tile([P, HEADS * P], bf16,
                                                        tag="pad", name=f"kpad{c}"))
                        ps = qkv_ps.tile([P, 512], f32, tag="qkvps",
                                         name=f"qkvps{n}_{c}")
                        nc.tensor.matmul(
                            ps[:], lhsT=ones_row[:],
                            rhs=bq_t[0:1, :],
                            start=True, stop=False)
                        for k in range(NKD):
                            nc.tensor.matmul(
                                ps[:],
                                lhsT=zT[:, k, c * P:(c + 1) * P],
                                rhs=wts[k][:],
                                start=False, stop=(k == NKD - 1))
                        if n < 2:
                            pvw = qpad_sb[c][:].rearrange("p (h d) -> p h d", d=P)
                            out_ap = pvw[:, (n % 2) * 8:(n % 2 + 1) * 8, 0:HD]
                        elif n < 4:
                            pvw = kpad_sb[c][:].rearrange("p (h d) -> p h d", d=P)
                            out_ap = pvw[:, (n % 2) * 8:(n % 2 + 1) * 8, 0:HD]
                        else:
                            vview = v_sb[c][:].rearrange("p (h e) -> p h e",
                                                         e=HD + 1)
                            out_ap = vview[:, (n % 2) * 8:(n % 2 + 1) * 8, 0:HD]
                        nc.vector.tensor_copy(out=out_ap, in_=ps[:])
                    if n == 1:
                        for c in range(NT):
                            rope_chunk(c, qpad_sb[c], cq_d, sq_d, qex_d, qT,
                                       "qpad_dbg")
                    if n == 3:
                        for c in range(NT):
                            rope_chunk(c, kpad_sb[c], ck_d, sk_d, kex_d, kT,
                                       "kpad_dbg")

            # ---------------- phase 3: attention --------------------------
            o_sb = [opool.tile([P, DIM], bf16, tag="o", name=f"o{m}")
                    for m in range(NT)]
            with ExitStack() as actx:
                et_pool = actx.enter_context(tc.tile_pool(name="etp", bufs=3 * NT))
                sc_ps = actx.enter_context(
                    tc.tile_pool(name="scps", bufs=2, space="PSUM"))
                av_ps = actx.enter_context(
                    tc.tile_pool(name="avps", bufs=2, space="PSUM"))
                for h in range(HEADS):
                    eT = []
                    for cts in range(NT):
                        ps = sc_ps.tile([P, TLOC], f32, tag="scps",
                                        name=f"scps{h}_{cts}")
                        for (off, size) in tq_splits:
                            nc.tensor.matmul(
                                ps[:, off:off + size],
                                lhsT=kT[0:HD + OH, h, cts * P:(cts + 1) * P],
                                rhs=qT[0:HD + OH, h, off:off + size],
                                start=True, stop=True)
                        et = et_pool.tile([P, TLOC], bf16, tag="eT",
                                          name=f"eT{h}_{cts}")
                        nc.scalar.activation(out=et[:], in_=ps[:], func=AF.Exp,
                                             bias=negbig_t[:], scale=1.0)
                        if DEBUG_DUMPS and h == 0:
                            nc.sync.dma_start(
                                out=dbg["et_dbg"][cts * P:(cts + 1) * P, :],
                                in_=et[:])
                        eT.append(et)
                    for m in range(NT):
                        po = av_ps.tile([P, HD + 1], f32, tag="avps",
                                        name=f"avps{h}_{m}")
                        for cts in range(NT):
                            nc.tensor.matmul(
                                po[:],
                                lhsT=eT[cts][:, m * P:(m + 1) * P],
                                rhs=v_sb[cts][:, h * (HD + 1):(h + 1) * (HD + 1)],
                                start=(cts == 0), stop=(cts == NT - 1))
                        rd = spool.tile([P, 1], f32, tag="sc", name=f"rd{h}_{m}")
                        if DEBUG_DUMPS and h == 0:
                            pocp = spool.tile([P, HD + 1], f32, tag="pocp",
                                              name=f"pocp{m}")
                            nc.vector.tensor_copy(out=pocp[:], in_=po[:])
                            nc.sync.dma_start(
                                out=dbg["po_dbg"][m * P:(m + 1) * P, :],
                                in_=pocp[:])
                        nc.vector.reciprocal(out=rd[:], in_=po[:, HD:HD + 1])
                        nc.vector.tensor_scalar_mul(
                            out=o_sb[m][:, h * HD:(h + 1) * HD],
                            in0=po[:, 0:HD], scalar1=rd[:])
            if DEBUG_DUMPS:
                for c in range(NT):
                    nc.sync.dma_start(out=dbg["v_dbg"][c * P:(c + 1) * P, :],
                                      in_=v_sb[c][:])
                    nc.sync.dma_start(out=dbg["o_dbg"][c * P:(c + 1) * P, :],
                                      in_=o_sb[c][:])

            # ------------- phase 4: out proj + residual + LN2 -------------
            x2_sb = []
            for m in range(NT):
                pe_transpose_into(
                    oT, [o_sb[m][:, f * P:(f + 1) * P] for f in range(NKD)],
                    slice(m * P, (m + 1) * P))
            with ExitStack() as octx:
                wo_pool = octx.enter_context(tc.tile_pool(name="wout", bufs=NKD))
                wo = {}
                for k in range(NKD):
                    wt = wo_pool.tile([P, DIM], bf16, tag="wout",
                                      name=f"wout{k}")
                    nc.gpsimd.dma_start(out=wt[:], in_=wout_d[k * P:(k + 1) * P, :])
                    wo[k] = wt
                op_ps = octx.enter_context(
                    tc.tile_pool(name="opps", bufs=4, space="PSUM"))
                with tc.tile_pool(name="z2pool", bufs=3) as z2pool:
                    for m in range(NT):
                        ps = [op_ps.tile([P, 512], f32, tag="opps",
                                         name=f"opps{m}_{n}") for n in range(2)]
                        for k in range(NKD):
                            for n in range(2):
                                nc.tensor.matmul(
                                    ps[n][:],
                                    lhsT=oT[:, k, m * P:(m + 1) * P],
                                    rhs=wo[k][:, n * 512:(n + 1) * 512],
                                    start=(k == 0), stop=(k == NKD - 1))
                        x2 = x2pool.tile([P, DIM], f32, tag="x2", name=f"x2_{m}")
                        for n in range(2):
                            nc.vector.tensor_add(
                                out=x2[:, n * 512:(n + 1) * 512], in0=ps[n][:],
                                in1=x_sb[m][:, n * 512:(n + 1) * 512])
                        x2_sb.append(x2)
                        z2 = z2pool.tile([P, DIM], bf16, tag="z2", name=f"z2_{m}")
                        ln_normalize(x2, z2)
                        if DEBUG_DUMPS:
                            nc.sync.dma_start(
                                out=dbg["z2_dbg"][m * P:(m + 1) * P, :], in_=z2[:])
                        pe_transpose_into(
                            z2T, [z2[:, f * P:(f + 1) * P] for f in range(NKD)],
                            slice(m * P, (m + 1) * P))
            p23.close()

            # ---------------- phase 5: MLP --------------------------------
            with ExitStack() as mctx:
                gh_pool = mctx.enter_context(tc.tile_pool(name="ghp", bufs=NKM))
                bm2pool = mctx.enter_context(tc.tile_pool(name="bm2pool", bufs=1))
                bm2_sb = bm2pool.tile([1, DIM], bf16, name="bm2_sb")
                nc.gpsimd.dma_start(out=bm2_sb[:],
                                    in_=bm2_d.rearrange("(o d) -> o d", o=1))
                wm1_pool = mctx.enter_context(tc.tile_pool(name="wm1p", bufs=10))
                m1_ps = mctx.enter_context(
                    tc.tile_pool(name="m1ps", bufs=2, space="PSUM"))
                ghT = []
                for mw in range(NKM // 8):
                    wts = {}
                    for k in range(NKD):
                        wt = wm1_pool.tile([P, DIM], bf16, tag="wm1",
                                           name=f"wm1_{mw}_{k}")
                        nc.gpsimd.dma_start(
                            out=wt[:],
                            in_=wm1_d[k * P:(k + 1) * P, mw * DIM:(mw + 1) * DIM])
                        wts[k] = wt
                    for mi in range(8):
                        m = mw * 8 + mi
                        pm = m1_ps.tile([P, TLOC], f32, tag="m1ps", name=f"m1ps{m}")
                        for k in range(NKD):
                            for (off, size) in tq_splits:
                                nc.tensor.matmul(
                                    pm[:, off:off + size],
                                    lhsT=wts[k][:, mi * P:(mi + 1) * P],
                                    rhs=z2T[:, k, off:off + size],
                                    start=(k == 0), stop=(k == NKD - 1))
                        gh = gh_pool.tile([P, TLOC], bf16, tag="ghT", name=f"ghT{m}")
                        nc.scalar.activation(out=gh[:], in_=pm[:],
                                             func=AF.Gelu_apprx_tanh,
                                             bias=bm1_sb[:, m:m + 1], scale=1.0)
                        ghT.append(gh)

                wm2_pool = mctx.enter_context(tc.tile_pool(name="wm2p", bufs=NKM))
                wm2 = {}
                for k in range(NKM):
                    wt = wm2_pool.tile([P, DIM], bf16, tag="wm2", name=f"wm2_{k}")
                    nc.gpsimd.dma_start(out=wt[:], in_=wm2_d[k * P:(k + 1) * P, :])
                    wm2[k] = wt
                m2_ps = mctx.enter_context(
                    tc.tile_pool(name="m2ps", bufs=4, space="PSUM"))
                with tc.tile_pool(name="ostage", bufs=2) as ostage:
                    for m in range(NT):
                        ps = [m2_ps.tile([P, 512], f32, tag="m2ps",
                                         name=f"m2ps{m}_{n}") for n in range(2)]
                        for n in range(2):
                            nc.tensor.matmul(
                                ps[n][:], lhsT=ones_row[:],
                                rhs=bm2_sb[0:1, n * 512:(n + 1) * 512],
                                start=True, stop=False)
                        for k in range(NKM):
                            for n in range(2):
                                nc.tensor.matmul(
                                    ps[n][:],
                                    lhsT=ghT[k][:, m * P:(m + 1) * P],
                                    rhs=wm2[k][:, n * 512:(n + 1) * 512],
                                    start=False, stop=(k == NKM - 1))
                        ot = ostage.tile([P, DIM], f32, tag="ost", name=f"ost{m}")
                        for n in range(2):
                            nc.vector.tensor_add(
                                out=ot[:, n * 512:(n + 1) * 512], in0=ps[n][:],
                                in1=x2_sb[m][:, n * 512:(n + 1) * 512])
                        nc.gpsimd.dma_start(out=out_d[m * P:(m + 1) * P, :],
                                            in_=ot[:])

    nc.compile()
    return nc


_NC_CACHE = {}


def _get_nc(TLOC):
    if TLOC not in _NC_CACHE:
        _NC_CACHE[TLOC] = _build(TLOC)
    return _NC_CACHE[TLOC]


# --------------------------------------------------------------------------
# host-side prep
# --------------------------------------------------------------------------

def _partition_contiguous(sizes, k):
    """Split list of segment sizes into <=k contiguous groups minimizing the
    max group total. Returns list of (start_seg, end_seg) pairs."""
    n = len(sizes)
    prefix = np.concatenate([[0], np.cumsum(sizes)])

    def feasible(cap):
        groups = 0
        i = 0
        while i < n:
            if sizes[i] > cap:
                return None
            j = i
            while j < n and prefix[j + 1] - prefix[i] <= cap:
                j += 1
            groups += 1
            i = j
        return groups

    lo, hi = int(max(sizes)), int(prefix[-1])
    while lo < hi:
        mid = (lo + hi) // 2
        g = feasible(mid)
        if g is not None and g <= k:
            hi = mid
        else:
            lo = mid + 1
    cap = lo
    out = []
    i = 0
    while i < n:
        j = i
        while j < n and prefix[j + 1] - prefix[i] <= cap:
            j += 1
        out.append((i, j))
        i = j
    return out


def _prep(inputs):
    x = np.asarray(inputs["x"], np.float32)[0]          # [T, D]
    cvec = np.asarray(inputs["c"], np.float32)          # [1, COND]
    cos = np.asarray(inputs["cos"], np.float32)         # [T, 32]
    sin = np.asarray(inputs["sin"], np.float32)
    seq = np.asarray(inputs["seq_idx"]).astype(np.int64)
    ln1_w = np.asarray(inputs["ln1_w"], np.float32)
    ln1_b = np.asarray(inputs["ln1_b"], np.float32)
    w_qkv = np.asarray(inputs["w_qkv"], np.float32)
    w_out = np.asarray(inputs["w_out"], np.float32)
    ln2_w = np.asarray(inputs["ln2_w"], np.float32)
    ln2_b = np.asarray(inputs["ln2_b"], np.float32)
    w_mlp1 = np.asarray(inputs["w_mlp1"], np.float32)
    b_mlp1 = np.asarray(inputs["b_mlp1"], np.float32)
    w_mlp2 = np.asarray(inputs["w_mlp2"], np.float32)
    b_mlp2 = np.asarray(inputs["b_mlp2"], np.float32)
    w_ada = np.asarray(inputs["w_ada"], np.float32)
    b_ada = np.asarray(inputs["b_ada"], np.float32)

    T = x.shape[0]

    ada = (cvec @ w_ada + b_ada)[0]                     # [6*DIM]
    sh_msa, sc_msa, g_msa, sh_mlp, sc_mlp, g_mlp = np.split(ada, 6)

    W1 = ln1_w * (1.0 + sc_msa)
    B1 = ln1_b * (1.0 + sc_msa) + sh_msa
    wqkv_f = (W1[:, None] * w_qkv).astype(BF16)
    bqkv_f = (B1 @ w_qkv).astype(BF16)
    wout_f = (w_out * g_msa[None, :]).astype(BF16)
    W2 = ln2_w * (1.0 + sc_mlp)
    B2 = ln2_b * (1.0 + sc_mlp) + sh_mlp
    wm1_f = (W2[:, None] * w_mlp1).astype(BF16)
    bm1_f = (b_mlp1 + B2 @ w_mlp1).astype(np.float32).reshape(MLP // P, P).T.copy()
    wm2_f = (w_mlp2 * g_mlp[None, :]).astype(BF16)
    bm2_f = (b_mlp2 * g_mlp).astype(BF16)

    # segment runs
    bnd = np.flatnonzero(np.diff(seq)) + 1
    seg_starts = np.concatenate([[0], bnd]).astype(int)
    seg_ends = np.concatenate([bnd, [T]]).astype(int)
    sizes = (seg_ends - seg_starts).astype(int)
    groups = _partition_contiguous(sizes, N_CORES)
    tok_ranges = [(seg_starts[a], seg_ends[b - 1]) for (a, b) in groups]
    while len(tok_ranges) < N_CORES:
        tok_ranges.append((T, T))                       # empty shard
    max_n = max(e - s for (s, e) in tok_ranges)
    TLOC = max(P, P * math.ceil(max_n / P))

    in_maps = []
    for (s, e) in tok_ranges:
        n = e - s
        x_loc = np.zeros((TLOC, DIM), np.float32)
        x_loc[:n] = x[s:e]
        ids = np.full(TLOC, PAD_ID, np.int64)
        ids[:n] = seq[s:e]
        oh = np.zeros((TLOC, OH), np.float32)
        oh[np.arange(TLOC), ids] = 1.0
        cos_loc = np.zeros((TLOC, HHD), np.float32)
        sin_loc = np.zeros((TLOC, HHD), np.float32)
        cos_loc[:n] = cos[s:e]
        sin_loc[:n] = sin[s:e]
        scale = 1.0 / math.sqrt(HD)
        in_maps.append({
            "x": x_loc,
            "cq": np.tile(scale * cos_loc, (1, HEADS)).astype(BF16),
            "sq": np.tile(scale * sin_loc, (1, HEADS)).astype(BF16),
            "ck": np.tile(cos_loc, (1, HEADS)).astype(BF16),
            "sk": np.tile(sin_loc, (1, HEADS)).astype(BF16),
            "qex": np.tile(BIG * oh, (1, HEADS)).astype(BF16),
            "kex": np.tile(oh, (1, HEADS)).astype(BF16),
            "wqkv": wqkv_f, "bqkv": bqkv_f,
            "wout": wout_f,
            "wm1": wm1_f, "bm1": bm1_f,
            "wm2": wm2_f, "bm2": bm2_f,
        })
    return in_maps, tok_ranges, TLOC, T


def kernel(**inputs) -> np.ndarray:
    in_maps, tok_ranges, TLOC, T = _prep(inputs)
    nc = _get_nc(TLOC)
    res = bass_utils.run_bass_kernel_spmd(nc, in_maps,
                                          core_ids=list(range(N_CORES)))
    out = np.empty((T, DIM), np.float32)
    for core, (s, e) in enumerate(tok_ranges):
        if e > s:
            out[s:e] = res.results[core]["out"][:e - s]
    return out[None]
